# revision 6
# baseline (speedup 1.0000x reference)
"""Trainium2 Bass kernel for nn_BubbleTransformer (2-layer attention-only
transformer, B=4 T=2048 D=1024 H=16, vocab 32000, logits of last token).

Distribution over 8 NeuronCores (one chip, LNC1):
  core c = 2*b + s  handles batch b = c//2 and query-half s = c%2.
  Tokens are PERMUTED per core so that the core's own queries always sit at
  positions 0-1023 (uniform SPMD instruction stream; all per-core variation
  is carried by input data: gather indices, positional rows, mask bias).
  K/V are computed for all 2048 tokens (replicated within the batch pair);
  Q / attention / out-projection only for the core's own 1024 queries.
  After layer 1 an 8-rank AllGather exchanges the per-core residual rows;
  after layer 2 a tiny AllGather shares the last-token rows; the vocab
  projection is sharded 8 x 4000 columns.

Compute is bf16 on the TensorEngine with fp32 PSUM accumulation; softmax is
exp on ScalarE with the causal mask folded into a per-slot exp bias
(0 or -30000) plus a small constant diagonal mask multiply on DVE.
"""

import os
from contextlib import ExitStack

import numpy as np
import ml_dtypes

import concourse.bass as bass
import concourse.tile as tile
from concourse import bacc, mybir
from concourse import bass_utils

F32 = mybir.dt.float32
BF16 = mybir.dt.bfloat16
I32 = mybir.dt.int32

V = 32000
D = 1024
H = 16
L = 2
T = 2048
B = 4
C = 32000
DH = 64
EPS = 1e-5

NCORES = 8
CSH = C // NCORES          # 4000 vocab columns per core
NCK = T // 128             # 16 token chunks
NDC = D // 128             # 8 d-chunks
E_LO, E_HI = 12, 16        # padded causal extents (in 128-key blocks)
NSLOT = E_LO + E_HI        # 28 schedule slots
NEG = -30000.0             # exp bias for fully-masked slots

# own-first permutation of the four 512-token superblocks
QSB_ORDER = {0: [0, 3, 1, 2], 1: [1, 2, 0, 3]}

DEBUG = bool(int(os.environ.get("BT_DEBUG", "0")))

_CACHE = {}


def _bf16(x):
    return np.asarray(x, np.float32).astype(ml_dtypes.bfloat16)


def _gathered_row(t, b):
    """Row of the global token (b, t) inside the layer-1 AllGather output."""
    q, r = t // 512, t % 512
    off = {0: 0, 3: 512, 1: 1024, 2: 1536}[q]
    return 2048 * b + off + r


def _host_prep(inputs):
    """Builds the per-core input maps (list of dict name->np array)."""
    tokens = np.asarray(inputs["tokens"]).astype(np.int32)      # [B, T]
    embw = _bf16(inputs["embed_W"])                             # [V, D]
    posW = np.asarray(inputs["pos_W"], np.float32)              # [T, D]
    ln_g = np.asarray(inputs["ln_g"], np.float32)               # [L, D]
    ln_b = np.asarray(inputs["ln_b"], np.float32)
    qkv_W = np.asarray(inputs["qkv_W"], np.float32)             # [L, D, 3D]
    qkv_b = np.asarray(inputs["qkv_b"], np.float32)             # [L, 3D]
    out_W = np.asarray(inputs["out_W"], np.float32)             # [L, D, D]
    out_b = np.asarray(inputs["out_b"], np.float32)             # [L, D]
    lnf_g = np.asarray(inputs["lnf_g"], np.float32)
    lnf_b = np.asarray(inputs["lnf_b"], np.float32)
    head_W = np.asarray(inputs["head_W"], np.float32)           # [D, C]
    head_b = np.asarray(inputs["head_b"], np.float32)           # [C]

    # fold the pre-attention LN affine into the qkv projection
    wqk = np.empty((L, D, 2 * D), ml_dtypes.bfloat16)
    wv = np.empty((L, D, D), ml_dtypes.bfloat16)
    bqkv = np.empty((L, 3 * D), np.float32)
    for l in range(L):
        weff = qkv_W[l] * ln_g[l][:, None]
        wqk[l] = _bf16(weff[:, :2 * D])
        wv[l] = _bf16(weff[:, 2 * D:])
        bqkv[l] = qkv_b[l] + ln_b[l] @ qkv_W[l]

    # Q/K biases as per-partition columns: bqk[l, p, j] ; j<8 -> Q chunk j,
    # j>=8 -> K chunk j-8
    bqk = np.empty((L, 128, 16), np.float32)
    for l in range(L):
        for j in range(8):
            bqk[l, :, j] = bqkv[l, 128 * j:128 * (j + 1)]
            bqk[l, :, 8 + j] = bqkv[l, D + 128 * j:D + 128 * (j + 1)]
    bv = np.tile(bqkv[:, None, 2 * D:], (1, 128, 1)).astype(np.float32)   # [L,128,D]
    bo = np.tile(out_b[:, None, :], (1, 128, 1)).astype(np.float32)       # [L,128,D]

    wo = _bf16(out_W)                                                     # [L, D, D]

    # diagonal causal masks, [128(k), 4(j), 512(q)]: 1 where q >= 128*j + k
    kk = np.arange(128)[:, None]
    qq = np.arange(512)[None, :]
    dmask = np.stack([(qq >= 128 * j + kk) for j in range(4)], axis=1)
    dmask = dmask.astype(ml_dtypes.bfloat16)                              # [128,4,512]

    ident = np.eye(128, dtype=np.float32)
    ones64 = np.ones((1, 64), np.float32)
    lnfg_t = np.tile(lnf_g[None, :], (4, 1)).astype(np.float32)
    lnfb_t = np.tile(lnf_b[None, :], (4, 1)).astype(np.float32)

    in_maps = []
    for c in range(NCORES):
        b, s = divmod(c, 2)
        order = QSB_ORDER[s]
        perm = np.concatenate([np.arange(512) + 512 * q for q in order])  # [T]

        tokidx = tokens[b][perm].reshape(NCK, 128).T.copy()               # [128,16]
        gidx = np.array([_gathered_row(t, b) for t in perm], np.int32)
        gidx = gidx.reshape(NCK, 128).T.copy()                            # [128,16]
        posw_c = posW[perm].copy()                                        # [T, D]

        # per-slot exp bias: 0 for diag/valid slots, NEG for future-key slots
        sb = np.zeros(NSLOT, np.float32)
        for u in range(E_LO):                                             # lo: q = order[0]
            if u >= 4 and order[u // 4] > order[0]:
                sb[u] = NEG
        for u in range(E_HI):                                             # hi: q = order[1]
            if (u // 4) != 1 and order[u // 4] > order[1]:
                sb[E_LO + u] = NEG
        sbias = np.tile(sb[None, :], (128, 1)).astype(np.float32)         # [128,28]

        headw_c = _bf16(head_W[:, CSH * c:CSH * (c + 1)])                 # [D, 4000]
        headb_c = np.tile(head_b[None, CSH * c:CSH * (c + 1)], (4, 1)).astype(np.float32)

        in_maps.append({
            "tokidx": tokidx, "gidx": gidx, "posw": posw_c,
            "embw": embw, "wqk": wqk, "wv": wv, "bqk": bqk, "bv": bv,
            "wo": wo, "bo": bo, "dmask": dmask, "sbias": sbias,
            "headw": headw_c, "headb": headb_c,
            "lnfg": lnfg_t, "lnfb": lnfb_t,
            "ident": ident, "ones64": ones64,
        })
    return in_maps


def _build():
    nc = bacc.Bacc("TRN2", target_bir_lowering=False, debug=False,
                   num_devices=NCORES)

    def din(name, shape, d):
        return nc.dram_tensor(name, shape, d, kind="ExternalInput").ap()

    tokidx = din("tokidx", [128, NCK], I32)
    gidx = din("gidx", [128, NCK], I32)
    posw = din("posw", [T, D], F32)
    embw = din("embw", [V, D], BF16)
    wqk = din("wqk", [L, D, 2 * D], BF16)
    wv = din("wv", [L, D, D], BF16)
    bqk = din("bqk", [L, 128, 16], F32)
    bv = din("bv", [L, 128, D], F32)
    wo = din("wo", [L, D, D], BF16)
    bo = din("bo", [L, 128, D], F32)
    dmask = din("dmask", [128, 4, 512], BF16)
    sbias = din("sbias", [128, NSLOT], F32)
    headw = din("headw", [D, CSH], BF16)
    headb = din("headb", [4, CSH], F32)
    lnfg = din("lnfg", [4, D], F32)
    lnfb = din("lnfb", [4, D], F32)
    ident = din("ident", [128, 128], F32)
    ones64 = din("ones64", [1, 64], F32)

    logits = nc.dram_tensor("logits", [4, CSH], F32, kind="ExternalOutput").ap()
    if DEBUG:
        dbg_h0 = nc.dram_tensor("dbg_h0", [T, D], F32, kind="ExternalOutput").ap()
        dbg_h1own = nc.dram_tensor("dbg_h1own", [1024, D], F32,
                                   kind="ExternalOutput").ap()
        dbg_x4 = nc.dram_tensor("dbg_x4", [4, D], F32, kind="ExternalOutput").ap()

    Exp = mybir.ActivationFunctionType.Exp
    Sqrt = mybir.ActivationFunctionType.Sqrt
    Alu = mybir.AluOpType

    with tile.TileContext(nc) as tc, ExitStack() as ctx:
        dram = ctx.enter_context(tc.tile_pool(name="dram", bufs=1, space="DRAM"))
        h0 = dram.tile([T, D], F32)
        agin1 = dram.tile([1024, D], F32)
        agout1 = dram.tile([NCORES * 1024, D], F32, addr_space="Shared")
        agin2 = dram.tile([1, D], F32)
        agout2 = dram.tile([NCORES, D], F32, addr_space="Shared")

        consts = ctx.enter_context(tc.tile_pool(name="consts", bufs=1))

        tok_sb = consts.tile([128, NCK], I32)
        nc.sync.dma_start(out=tok_sb[:], in_=tokidx[:])
        gidx_sb = consts.tile([128, NCK], I32)
        nc.sync.dma_start(out=gidx_sb[:], in_=gidx[:])
        dmask_sb = consts.tile([128, 4, 512], BF16)
        nc.sync.dma_start(out=dmask_sb[:], in_=dmask[:])
        sbias_sb = consts.tile([128, NSLOT], F32)
        nc.sync.dma_start(out=sbias_sb[:], in_=sbias[:])
        ones64_sb = consts.tile([1, 64], F32)
        nc.sync.dma_start(out=ones64_sb[:], in_=ones64[:])
        eps_sb = consts.tile([128, 1], F32)
        nc.vector.memset(eps_sb[:], EPS)

        def layernorm_tile(pool, h_tile, xn_tile, p=128):
            """xn = (h - mean) * rsqrt(var + eps), fp32 -> bf16, [p, D]."""
            stats = pool.tile([128, 2, 6], F32, name="stats")
            nc.vector.bn_stats(out=stats[:p, 0, :], in_=h_tile[:p, 0:512])
            nc.vector.bn_stats(out=stats[:p, 1, :], in_=h_tile[:p, 512:1024])
            mv = pool.tile([128, 2], F32, name="mv")
            nc.vector.bn_aggr(out=mv[:p], in_=stats[:p])
            rstd = pool.tile([128, 1], F32, name="rstd")
            nc.scalar.activation(out=rstd[:p], in_=mv[:p, 1:2], func=Sqrt,
                                 bias=eps_sb[:p], scale=1.0)
            nc.vector.reciprocal(out=rstd[:p], in_=rstd[:p])
            nc.vector.tensor_scalar(out=xn_tile[:p], in0=h_tile[:p],
                                    scalar1=mv[:p, 0:1], scalar2=rstd[:p],
                                    op0=Alu.subtract, op1=Alu.mult)

        for li in range(L):
            lyr = ExitStack()
            with lyr:
                lw = lyr.enter_context(
                    tc.tile_pool(name=f"lw{li}", bufs=1))
                wqk_sb = lw.tile([128, NDC, 2 * D], BF16, name="wqk_sb")
                for dc in range(NDC):
                    nc.sync.dma_start(out=wqk_sb[:, dc, :],
                                      in_=wqk[li, 128 * dc:128 * (dc + 1), :])
                bqk_sb = lw.tile([128, 16], F32, name="bqk_sb")
                nc.sync.dma_start(out=bqk_sb[:], in_=bqk[li])
                bv_sb = lw.tile([128, D], F32, name="bv_sb")
                nc.sync.dma_start(out=bv_sb[:], in_=bv[li])
                bo_sb = lw.tile([128, D], F32, name="bo_sb")
                nc.sync.dma_start(out=bo_sb[:], in_=bo[li])

                xnT = lw.tile([128, NDC, T], BF16, name="xnT")
                V_sb = lw.tile([128, NCK, H, DH + 1], BF16, name="V_sb")
                oT = lw.tile([128, NDC, 1024], BF16, name="oT")

                # ---------- phase A: residual source + LN + transpose ------
                with tc.tile_pool(name=f"lnA{li}", bufs=2) as lnp:
                    for ck in range(NCK):
                        h_tile = lnp.tile([128, D], F32, name="h_tile")
                        if li == 0:
                            emb = lnp.tile([128, D], BF16, name="emb")
                            nc.gpsimd.indirect_dma_start(
                                out=emb[:], out_offset=None, in_=embw[:],
                                in_offset=bass.IndirectOffsetOnAxis(
                                    ap=tok_sb[:, ck:ck + 1], axis=0))
                            pos = lnp.tile([128, D], F32, name="pos")
                            nc.sync.dma_start(
                                out=pos[:], in_=posw[128 * ck:128 * (ck + 1), :])
                            nc.vector.tensor_add(out=h_tile[:], in0=emb[:],
                                                 in1=pos[:])
                            nc.sync.dma_start(
                                out=h0[128 * ck:128 * (ck + 1), :], in_=h_tile[:])
                            if DEBUG:
                                nc.sync.dma_start(
                                    out=dbg_h0[128 * ck:128 * (ck + 1), :],
                                    in_=h_tile[:])
                        elif ck < 8:
                            nc.sync.dma_start(
                                out=h_tile[:],
                                in_=agin1[128 * ck:128 * (ck + 1), :])
                        else:
                            nc.gpsimd.indirect_dma_start(
                                out=h_tile[:], out_offset=None, in_=agout1[:],
                                in_offset=bass.IndirectOffsetOnAxis(
                                    ap=gidx_sb[:, ck:ck + 1], axis=0))
                        xn = lnp.tile([128, D], BF16, name="xn")
                        layernorm_tile(lnp, h_tile, xn)
                        for dc in range(NDC):
                            nc.sync.dma_start_transpose(
                                out=xnT[:, dc, 128 * ck:128 * (ck + 1)],
                                in_=xn[:, 128 * dc:128 * (dc + 1)])

                # ---------- phase B1: V for all tokens ----------
                with tc.tile_pool(name=f"wv{li}", bufs=1) as wvp, \
                     tc.tile_pool(name=f"psB{li}", bufs=1, space="PSUM") as psB:
                    wv_sb = wvp.tile([128, NDC, D], BF16, name="wv_sb")
                    for dc in range(NDC):
                        nc.sync.dma_start(out=wv_sb[:, dc, :],
                                          in_=wv[li, 128 * dc:128 * (dc + 1), :])
                    for ck in range(NCK):
                        for half in range(2):
                            vps = psB.tile([128, 512], F32, name="vps", bufs=2)
                            for dc in range(NDC):
                                nc.tensor.matmul(
                                    vps[:],
                                    lhsT=xnT[:, dc, 128 * ck:128 * (ck + 1)],
                                    rhs=wv_sb[:, dc, 512 * half:512 * (half + 1)],
                                    start=(dc == 0), stop=(dc == NDC - 1))
                            nc.vector.scalar_tensor_tensor(
                                out=V_sb[:, ck, 8 * half:8 * (half + 1), 0:DH],
                                in0=vps[:].rearrange("p (h d) -> p h d", h=8),
                                scalar=1.0,
                                in1=bv_sb[:, 512 * half:512 * (half + 1)].rearrange(
                                    "p (h d) -> p h d", h=8),
                                op0=Alu.mult, op1=Alu.add)
                        nc.vector.memset(V_sb[:, ck, :, DH:DH + 1], 1.0)

                # ---------- phases B2+C: per pair K,Q then attention -------
                with tc.tile_pool(name=f"kq{li}", bufs=2) as kqp, \
                     tc.tile_pool(name=f"pt{li}", bufs=3) as pp, \
                     tc.tile_pool(name=f"psC{li}", bufs=1, space="PSUM") as psC:
                    for p in range(NDC):
                        kt = kqp.tile([128, T], BF16, name="kt")
                        for ts in range(4):
                            kps = psC.tile([128, 512], F32, name="kqps", bufs=2)
                            for dc in range(NDC):
                                nc.tensor.matmul(
                                    kps[:],
                                    lhsT=wqk_sb[:, dc, D + 128 * p:D + 128 * (p + 1)],
                                    rhs=xnT[:, dc, 512 * ts:512 * (ts + 1)],
                                    start=(dc == 0), stop=(dc == NDC - 1))
                            nc.scalar.add(out=kt[:, 512 * ts:512 * (ts + 1)],
                                          in_=kps[:], add=bqk_sb[:, 8 + p:9 + p])
                        qt = kqp.tile([128, 1024], BF16, name="qt")
                        for ts in range(2):
                            qps = psC.tile([128, 512], F32, name="kqps", bufs=2)
                            for dc in range(NDC):
                                nc.tensor.matmul(
                                    qps[:],
                                    lhsT=wqk_sb[:, dc, 128 * p:128 * (p + 1)],
                                    rhs=xnT[:, dc, 512 * ts:512 * (ts + 1)],
                                    start=(dc == 0), stop=(dc == NDC - 1))
                            nc.scalar.add(out=qt[:, 512 * ts:512 * (ts + 1)],
                                          in_=qps[:], add=bqk_sb[:, p:p + 1])

                        for qsb in range(2):
                            E = E_LO if qsb == 0 else E_HI
                            base = 0 if qsb == 0 else E_LO
                            qoff = 512 * qsb
                            o_psA = psC.tile([65, 512], F32, name="o_psA")
                            o_psB = psC.tile([65, 512], F32, name="o_psB")
                            o_ps = [o_psA, o_psB]
                            for u in range(E):
                                sps = psC.tile([128, 1024], F32, name="sps",
                                               bufs=2)
                                for j in range(2):  # head A | head B
                                    nc.tensor.matmul(
                                        sps[:, 512 * j:512 * (j + 1)],
                                        lhsT=kt[64 * j:64 * (j + 1),
                                                128 * u:128 * (u + 1)],
                                        rhs=qt[64 * j:64 * (j + 1),
                                               qoff:qoff + 512],
                                        start=True, stop=True,
                                        tile_position=(64 * j, 0))
                                P = pp.tile([128, 1024], BF16, name="P")
                                nc.scalar.activation(
                                    out=P[:], in_=sps[:], func=Exp,
                                    bias=sbias_sb[:, base + u:base + u + 1],
                                    scale=0.125)
                                dslot = u if qsb == 0 else u - 4
                                if 0 <= dslot < 4:
                                    for j in range(2):
                                        nc.vector.tensor_mul(
                                            out=P[:, 512 * j:512 * (j + 1)],
                                            in0=P[:, 512 * j:512 * (j + 1)],
                                            in1=dmask_sb[:, dslot, :])
                                for j in range(2):
                                    nc.tensor.matmul(
                                        o_ps[j][:],
                                        lhsT=V_sb[:, u, 2 * p + j, :],
                                        rhs=P[:, 512 * j:512 * (j + 1)],
                                        start=(u == 0), stop=(u == E - 1))
                            for j in range(2):
                                recip = pp.tile([1, 512], F32, name="recip")
                                nc.vector.reciprocal(out=recip[:],
                                                     in_=o_ps[j][64:65, :])
                                rb = psC.tile([64, 512], F32, name="kqps",
                                              bufs=2)
                                nc.tensor.matmul(rb[:], lhsT=ones64_sb[:],
                                                 rhs=recip[:], start=True,
                                                 stop=True)
                                oU = pp.tile([64, 512], BF16, name="oU")
                                nc.vector.tensor_copy(out=oU[:],
                                                      in_=o_ps[j][0:64, :])
                                nc.vector.tensor_mul(
                                    out=oT[64 * j:64 * (j + 1), p,
                                           qoff:qoff + 512],
                                    in0=oU[:], in1=rb[:])

                # ---------- phase D: out projection + residual ----------
                with tc.tile_pool(name=f"oD{li}", bufs=2) as dpool, \
                     tc.tile_pool(name=f"woD{li}", bufs=1) as wop, \
                     tc.tile_pool(name=f"psD{li}", bufs=1, space="PSUM") as psD:
                    wo_sb = wop.tile([128, NDC, D], BF16, name="wo_sb")
                    for dc in range(NDC):
                        nc.sync.dma_start(out=wo_sb[:, dc, :],
                                          in_=wo[li, 128 * dc:128 * (dc + 1), :])
                    for qb in range(8):
                        h_tile = dpool.tile([128, D], F32, name="h_res")
                        if li == 0:
                            nc.sync.dma_start(
                                out=h_tile[:],
                                in_=h0[128 * qb:128 * (qb + 1), :])
                        else:
                            nc.sync.dma_start(
                                out=h_tile[:],
                                in_=agin1[128 * qb:128 * (qb + 1), :])
                        hn = dpool.tile([128, D], F32, name="hn")
                        for half in range(2):
                            ops = psD.tile([128, 512], F32, name="ops", bufs=2)
                            for dc in range(NDC):
                                nc.tensor.matmul(
                                    ops[:],
                                    lhsT=oT[:, dc, 128 * qb:128 * (qb + 1)],
                                    rhs=wo_sb[:, dc, 512 * half:512 * (half + 1)],
                                    start=(dc == 0), stop=(dc == NDC - 1))
                            nc.vector.scalar_tensor_tensor(
                                out=hn[:, 512 * half:512 * (half + 1)],
                                in0=ops[:], scalar=1.0,
                                in1=h_tile[:, 512 * half:512 * (half + 1)],
                                op0=Alu.mult, op1=Alu.add)
                            nc.vector.tensor_add(
                                out=hn[:, 512 * half:512 * (half + 1)],
                                in0=hn[:, 512 * half:512 * (half + 1)],
                                in1=bo_sb[:, 512 * half:512 * (half + 1)])
                        if li == 0:
                            nc.sync.dma_start(
                                out=agin1[128 * qb:128 * (qb + 1), :],
                                in_=hn[:])
                        elif qb == 7:
                            nc.sync.dma_start(out=agin2[0:1, :],
                                              in_=hn[127:128, :])

            if li == 0:
                nc.gpsimd.collective_compute(
                    "AllGather", Alu.bypass,
                    replica_groups=[list(range(NCORES))],
                    ins=[agin1.opt()], outs=[agout1.opt()])
                if DEBUG:
                    nc.sync.dma_start(out=dbg_h1own[:], in_=agin1[:])
            else:
                nc.gpsimd.collective_compute(
                    "AllGather", Alu.bypass,
                    replica_groups=[list(range(NCORES))],
                    ins=[agin2.opt()], outs=[agout2.opt()])

        # ---------- head: final LN + vocab-sharded projection ----------
        with tc.tile_pool(name="hd", bufs=1) as hd, \
             tc.tile_pool(name="hdw", bufs=2) as hdw:
            x4 = hd.tile([4, D], F32)
            for i in range(4):
                nc.sync.dma_start(out=x4[i:i + 1, :],
                                  in_=agout2[2 * i:2 * i + 1, :])
            if DEBUG:
                nc.sync.dma_start(out=dbg_x4[:], in_=x4[:])
            xnf = hd.tile([4, D], F32)
            layernorm_tile(hd, x4, xnf, p=4)
            lnfg_sb = hd.tile([4, D], F32)
            nc.sync.dma_start(out=lnfg_sb[:], in_=lnfg[:])
            lnfb_sb = hd.tile([4, D], F32)
            nc.sync.dma_start(out=lnfb_sb[:], in_=lnfb[:])
            nc.vector.tensor_mul(out=xnf[:], in0=xnf[:], in1=lnfg_sb[:])
            nc.vector.tensor_add(out=xnf[:], in0=xnf[:], in1=lnfb_sb[:])

            ident_sb = hd.tile([128, 128], F32)
            nc.sync.dma_start(out=ident_sb[:], in_=ident[:])
            xhT = hd.tile([128, NDC, 4], BF16)
            with tc.tile_pool(name="psT", bufs=1, space="PSUM") as psT:
                for dc in range(NDC):
                    tps = psT.tile([128, 4], F32, name="tps", bufs=2)
                    nc.tensor.transpose(out=tps[:],
                                        in_=xnf[:, 128 * dc:128 * (dc + 1)],
                                        identity=ident_sb[0:4, 0:4])
                    nc.vector.tensor_copy(out=xhT[:, dc, :], in_=tps[:])

            headb_sb = hd.tile([4, CSH], F32)
            nc.sync.dma_start(out=headb_sb[:], in_=headb[:])
            with tc.tile_pool(name="psL", bufs=1, space="PSUM") as psL:
                lps = [psL.tile([4, 500], F32, name=f"lps{nb}")
                       for nb in range(8)]
                for dc in range(NDC):
                    hw = hdw.tile([128, CSH], BF16, name="hw")
                    nc.sync.dma_start(out=hw[:],
                                      in_=headw[128 * dc:128 * (dc + 1), :])
                    for nb in range(8):
                        nc.tensor.matmul(lps[nb][:], lhsT=xhT[:, dc, :],
                                         rhs=hw[:, 500 * nb:500 * (nb + 1)],
                                         start=(dc == 0), stop=(dc == NDC - 1))
                for nb in range(8):
                    lsb = hdw.tile([4, 500], F32, name="lsb")
                    nc.vector.tensor_add(out=lsb[:], in0=lps[nb][:],
                                         in1=headb_sb[:, 500 * nb:500 * (nb + 1)])
                    nc.sync.dma_start(out=logits[:, 500 * nb:500 * (nb + 1)],
                                      in_=lsb[:])

    nc.compile()
    return nc


def get_nc():
    if "nc" not in _CACHE:
        _CACHE["nc"] = _build()
    return _CACHE["nc"]


def run_spmd(in_maps):
    nc = get_nc()
    return bass_utils.run_bass_kernel_spmd(nc, in_maps, core_ids=list(range(NCORES)))


def kernel(**inputs) -> np.ndarray:
    in_maps = _host_prep(inputs)
    res = run_spmd(in_maps)
    out = np.empty((B, C), np.float32)
    for c in range(NCORES):
        out[:, CSH * c:CSH * (c + 1)] = res.results[c]["logits"]
    _CACHE["last_results"] = res
    return out


# revision 12
# speedup vs baseline: 15335.8942x; 15335.8942x over previous
"""Trainium2 Bass kernel for nn_BubbleTransformer (2-layer attention-only
transformer, B=4 T=2048 D=1024 H=16, vocab 32000, logits of last token).

Distribution over 8 NeuronCores (one chip, LNC1):
  core c = 2*b + s  handles batch b = c//2 and query-half s = c%2.
  Tokens are PERMUTED per core so that the core's own queries always sit at
  positions 0-1023 (uniform SPMD instruction stream; all per-core variation
  is carried by input data: gather indices, positional rows, mask bias).
  K/V are computed for all 2048 tokens (replicated within the batch pair);
  Q / attention / out-projection only for the core's own 1024 queries.
  After layer 1 an 8-rank AllGather exchanges the per-core residual rows;
  after layer 2 a tiny AllGather shares the last-token rows; the vocab
  projection is sharded 8 x 4000 columns.

Compute is bf16 on the TensorEngine with fp32 PSUM accumulation; softmax is
exp on ScalarE with the causal mask folded into a per-slot exp bias
(0 or -30000) plus a small constant diagonal mask multiply on DVE.
"""

import os
from contextlib import ExitStack

import numpy as np
import ml_dtypes

import concourse.bass as bass
import concourse.tile as tile
from concourse import bacc, mybir
from concourse import bass_utils

F32 = mybir.dt.float32
BF16 = mybir.dt.bfloat16
I32 = mybir.dt.int32

V = 32000
D = 1024
H = 16
L = 2
T = 2048
B = 4
C = 32000
DH = 64
EPS = 1e-5

NCORES = 8
CSH = C // NCORES          # 4000 vocab columns per core
NCK = T // 128             # 16 token chunks
NDC = D // 128             # 8 d-chunks
E_LO, E_HI = 12, 16        # padded causal extents (in 128-key blocks)
NSLOT = E_LO + E_HI        # 28 schedule slots
NEG = -30000.0             # exp bias for fully-masked slots

# own-first permutation of the four 512-token superblocks
QSB_ORDER = {0: [0, 3, 1, 2], 1: [1, 2, 0, 3]}

DEBUG = bool(int(os.environ.get("BT_DEBUG", "0")))

_CACHE = {}


def _bf16(x):
    return np.asarray(x, np.float32).astype(ml_dtypes.bfloat16)


def _gathered_row(t, b):
    """Row of the global token (b, t) inside the layer-1 AllGather output."""
    q, r = t // 512, t % 512
    off = {0: 0, 3: 512, 1: 1024, 2: 1536}[q]
    return 2048 * b + off + r


def _host_prep(inputs):
    """Builds the per-core input maps (list of dict name->np array)."""
    tokens = np.asarray(inputs["tokens"]).astype(np.int32)      # [B, T]
    embw = _bf16(inputs["embed_W"])                             # [V, D]
    posW = np.asarray(inputs["pos_W"], np.float32)              # [T, D]
    ln_g = np.asarray(inputs["ln_g"], np.float32)               # [L, D]
    ln_b = np.asarray(inputs["ln_b"], np.float32)
    qkv_W = np.asarray(inputs["qkv_W"], np.float32)             # [L, D, 3D]
    qkv_b = np.asarray(inputs["qkv_b"], np.float32)             # [L, 3D]
    out_W = np.asarray(inputs["out_W"], np.float32)             # [L, D, D]
    out_b = np.asarray(inputs["out_b"], np.float32)             # [L, D]
    lnf_g = np.asarray(inputs["lnf_g"], np.float32)
    lnf_b = np.asarray(inputs["lnf_b"], np.float32)
    head_W = np.asarray(inputs["head_W"], np.float32)           # [D, C]
    head_b = np.asarray(inputs["head_b"], np.float32)           # [C]

    # fold the pre-attention LN affine into the qkv projection
    wqk = np.empty((L, D, 2 * D), ml_dtypes.bfloat16)
    wv = np.empty((L, D, D), ml_dtypes.bfloat16)
    bqkv = np.empty((L, 3 * D), np.float32)
    for l in range(L):
        weff = qkv_W[l] * ln_g[l][:, None]
        wqk[l] = _bf16(weff[:, :2 * D])
        wv[l] = _bf16(weff[:, 2 * D:])
        bqkv[l] = qkv_b[l] + ln_b[l] @ qkv_W[l]

    # Q/K biases as per-partition columns: bqk[l, p, j] ; j<8 -> Q chunk j,
    # j>=8 -> K chunk j-8
    bqk = np.empty((L, 128, 16), np.float32)
    for l in range(L):
        for j in range(8):
            bqk[l, :, j] = bqkv[l, 128 * j:128 * (j + 1)]
            bqk[l, :, 8 + j] = bqkv[l, D + 128 * j:D + 128 * (j + 1)]
    bv = np.tile(bqkv[:, None, 2 * D:], (1, 128, 1)).astype(np.float32)   # [L,128,D]
    bo = np.tile(out_b[:, None, :], (1, 128, 1)).astype(np.float32)       # [L,128,D]

    wo = _bf16(out_W)                                                     # [L, D, D]

    # diagonal causal masks, [128(k), 4(j), 512(q)]: 1 where q >= 128*j + k
    kk = np.arange(128)[:, None]
    qq = np.arange(512)[None, :]
    dmask = np.stack([(qq >= 128 * j + kk) for j in range(4)], axis=1)
    dmask = dmask.astype(ml_dtypes.bfloat16)                              # [128,4,512]

    ident = np.eye(128, dtype=np.float32)
    ones64 = np.ones((1, 64), np.float32)
    lnfg_t = np.tile(lnf_g[None, :], (4, 1)).astype(np.float32)
    lnfb_t = np.tile(lnf_b[None, :], (4, 1)).astype(np.float32)

    in_maps = []
    for c in range(NCORES):
        b, s = divmod(c, 2)
        order = QSB_ORDER[s]
        perm = np.concatenate([np.arange(512) + 512 * q for q in order])  # [T]

        tokidx = tokens[b][perm].reshape(NCK, 128).T.copy()               # [128,16]
        gidx = np.array([_gathered_row(t, b) for t in perm], np.int32)
        gidx = gidx.reshape(NCK, 128).T.copy()                            # [128,16]
        posw_c = posW[perm].copy()                                        # [T, D]

        # per-slot exp bias: 0 for diag/valid slots, NEG for future-key slots
        sb = np.zeros(NSLOT, np.float32)
        for u in range(E_LO):                                             # lo: q = order[0]
            if u >= 4 and order[u // 4] > order[0]:
                sb[u] = NEG
        for u in range(E_HI):                                             # hi: q = order[1]
            if (u // 4) != 1 and order[u // 4] > order[1]:
                sb[E_LO + u] = NEG
        sbias = np.tile(sb[None, :], (128, 1)).astype(np.float32)         # [128,28]

        headw_c = _bf16(head_W[:, CSH * c:CSH * (c + 1)])                 # [D, 4000]
        headb_c = np.tile(head_b[None, CSH * c:CSH * (c + 1)], (4, 1)).astype(np.float32)

        in_maps.append({
            "tokidx": tokidx, "gidx": gidx, "posw": posw_c,
            "embw": embw, "wqk": wqk, "wv": wv, "bqk": bqk, "bv": bv,
            "wo": wo, "bo": bo, "dmask": dmask, "sbias": sbias,
            "headw": headw_c, "headb": headb_c,
            "lnfg": lnfg_t, "lnfb": lnfb_t,
            "ident": ident, "ones64": ones64,
        })
    return in_maps


def _build(repeats=1):
    nc = bacc.Bacc("TRN2", target_bir_lowering=False, debug=False,
                   num_devices=NCORES)

    def din(name, shape, d):
        return nc.dram_tensor(name, shape, d, kind="ExternalInput").ap()

    tokidx = din("tokidx", [128, NCK], I32)
    gidx = din("gidx", [128, NCK], I32)
    posw = din("posw", [T, D], F32)
    embw = din("embw", [V, D], BF16)
    wqk = din("wqk", [L, D, 2 * D], BF16)
    wv = din("wv", [L, D, D], BF16)
    bqk = din("bqk", [L, 128, 16], F32)
    bv = din("bv", [L, 128, D], F32)
    wo = din("wo", [L, D, D], BF16)
    bo = din("bo", [L, 128, D], F32)
    dmask = din("dmask", [128, 4, 512], BF16)
    sbias = din("sbias", [128, NSLOT], F32)
    headw = din("headw", [D, CSH], BF16)
    headb = din("headb", [4, CSH], F32)
    lnfg = din("lnfg", [4, D], F32)
    lnfb = din("lnfb", [4, D], F32)
    ident = din("ident", [128, 128], F32)
    ones64 = din("ones64", [1, 64], F32)

    logits = nc.dram_tensor("logits", [4, CSH], F32, kind="ExternalOutput").ap()
    if DEBUG:
        dbg_h0 = nc.dram_tensor("dbg_h0", [T, D], F32, kind="ExternalOutput").ap()
        dbg_h1own = nc.dram_tensor("dbg_h1own", [1024, D], F32,
                                   kind="ExternalOutput").ap()
        dbg_x4 = nc.dram_tensor("dbg_x4", [4, D], F32, kind="ExternalOutput").ap()

    Exp = mybir.ActivationFunctionType.Exp
    Sqrt = mybir.ActivationFunctionType.Sqrt
    Alu = mybir.AluOpType

    with tile.TileContext(nc) as tc, ExitStack() as ctx:
        dram = ctx.enter_context(tc.tile_pool(name="dram", bufs=1, space="DRAM"))
        h0 = dram.tile([T, D], F32)
        agin1s, agout1s, agin2s, agout2s = [], [], [], []
        for r in range(repeats):
            agin1s.append(dram.tile([1024, D], F32, name=f"agin1_{r}"))
            agout1s.append(dram.tile([NCORES * 1024, D], F32,
                                     addr_space="Shared", name=f"agout1_{r}"))
            agin2s.append(dram.tile([1, D], F32, name=f"agin2_{r}"))
            agout2s.append(dram.tile([NCORES, D], F32,
                                     addr_space="Shared", name=f"agout2_{r}"))

        consts = ctx.enter_context(tc.tile_pool(name="consts", bufs=1))

        tok_sb = consts.tile([128, NCK], I32)
        nc.sync.dma_start(out=tok_sb[:], in_=tokidx[:])
        gidx_sb = consts.tile([128, NCK], I32)
        nc.sync.dma_start(out=gidx_sb[:], in_=gidx[:])
        dmask_sb = consts.tile([128, 4, 512], BF16)
        nc.sync.dma_start(out=dmask_sb[:], in_=dmask[:])
        sbias_sb = consts.tile([128, NSLOT], F32)
        nc.sync.dma_start(out=sbias_sb[:], in_=sbias[:])
        ones64_sb = consts.tile([1, 64], F32)
        nc.sync.dma_start(out=ones64_sb[:], in_=ones64[:])
        eps_sb = consts.tile([128, 1], F32)
        nc.vector.memset(eps_sb[:], EPS)

        def layernorm_tile(pool, h_tile, xn_tile, p=128):
            """xn = (h - mean) * rsqrt(var + eps), fp32 -> bf16, [p, D]."""
            stats = pool.tile([128, 2, 6], F32, name="stats")
            nc.vector.bn_stats(out=stats[:p, 0, :], in_=h_tile[:p, 0:512])
            nc.vector.bn_stats(out=stats[:p, 1, :], in_=h_tile[:p, 512:1024])
            mv = pool.tile([128, 2], F32, name="mv")
            nc.vector.bn_aggr(out=mv[:p], in_=stats[:p])
            rstd = pool.tile([128, 1], F32, name="rstd")
            nc.scalar.activation(out=rstd[:p], in_=mv[:p, 1:2], func=Sqrt,
                                 bias=eps_sb[:p], scale=1.0)
            nc.vector.reciprocal(out=rstd[:p], in_=rstd[:p])
            nc.vector.tensor_scalar(out=xn_tile[:p], in0=h_tile[:p],
                                    scalar1=mv[:p, 0:1], scalar2=rstd[:p],
                                    op0=Alu.subtract, op1=Alu.mult)

        for rep, li in [(r, l) for r in range(repeats) for l in range(L)]:
            rl = f"{rep}_{li}"
            agin1, agout1 = agin1s[rep], agout1s[rep]
            agin2, agout2 = agin2s[rep], agout2s[rep]
            lyr = ExitStack()
            with lyr:
                lw = lyr.enter_context(
                    tc.tile_pool(name=f"lw{rl}", bufs=1))
                wqk_sb = lw.tile([128, NDC, 2 * D], BF16, name="wqk_sb")
                for dc in range(NDC):
                    nc.sync.dma_start(out=wqk_sb[:, dc, :],
                                      in_=wqk[li, 128 * dc:128 * (dc + 1), :])
                bqk_sb = lw.tile([128, 16], F32, name="bqk_sb")
                nc.sync.dma_start(out=bqk_sb[:], in_=bqk[li])
                bv_sb = lw.tile([128, D], F32, name="bv_sb")
                nc.sync.dma_start(out=bv_sb[:], in_=bv[li])
                bo_sb = lw.tile([128, D], F32, name="bo_sb")
                nc.sync.dma_start(out=bo_sb[:], in_=bo[li])

                xnT = lw.tile([128, NDC, T], BF16, name="xnT")
                V_sb = lw.tile([128, NCK, H, DH + 1], BF16, name="V_sb")
                oT = lw.tile([128, NDC, 1024], BF16, name="oT")

                # ---------- phase A: residual source + LN + transpose ------
                with tc.tile_pool(name=f"lnA{rl}", bufs=2) as lnp:
                    for ck in range(NCK):
                        h_tile = lnp.tile([128, D], F32, name="h_tile")
                        if li == 0:
                            emb = lnp.tile([128, D], BF16, name="emb")
                            nc.gpsimd.indirect_dma_start(
                                out=emb[:], out_offset=None, in_=embw[:],
                                in_offset=bass.IndirectOffsetOnAxis(
                                    ap=tok_sb[:, ck:ck + 1], axis=0))
                            pos = lnp.tile([128, D], F32, name="pos")
                            nc.sync.dma_start(
                                out=pos[:], in_=posw[128 * ck:128 * (ck + 1), :])
                            nc.vector.tensor_add(out=h_tile[:], in0=emb[:],
                                                 in1=pos[:])
                            nc.sync.dma_start(
                                out=h0[128 * ck:128 * (ck + 1), :], in_=h_tile[:])
                            if DEBUG:
                                nc.sync.dma_start(
                                    out=dbg_h0[128 * ck:128 * (ck + 1), :],
                                    in_=h_tile[:])
                        elif ck < 8:
                            nc.sync.dma_start(
                                out=h_tile[:],
                                in_=agin1[128 * ck:128 * (ck + 1), :])
                        else:
                            nc.gpsimd.indirect_dma_start(
                                out=h_tile[:], out_offset=None, in_=agout1[:],
                                in_offset=bass.IndirectOffsetOnAxis(
                                    ap=gidx_sb[:, ck:ck + 1], axis=0))
                        xn = lnp.tile([128, D], BF16, name="xn")
                        layernorm_tile(lnp, h_tile, xn)
                        for dc in range(NDC):
                            nc.sync.dma_start_transpose(
                                out=xnT[:, dc, 128 * ck:128 * (ck + 1)],
                                in_=xn[:, 128 * dc:128 * (dc + 1)])

                # ---------- phase B1: V for all tokens ----------
                with tc.tile_pool(name=f"wv{rl}", bufs=1) as wvp, \
                     tc.tile_pool(name=f"psB{rl}", bufs=1, space="PSUM") as psB:
                    wv_sb = wvp.tile([128, NDC, D], BF16, name="wv_sb")
                    for dc in range(NDC):
                        nc.sync.dma_start(out=wv_sb[:, dc, :],
                                          in_=wv[li, 128 * dc:128 * (dc + 1), :])
                    for ck in range(NCK):
                        for half in range(2):
                            vps = psB.tile([128, 512], F32, name="vps", bufs=2)
                            for dc in range(NDC):
                                nc.tensor.matmul(
                                    vps[:],
                                    lhsT=xnT[:, dc, 128 * ck:128 * (ck + 1)],
                                    rhs=wv_sb[:, dc, 512 * half:512 * (half + 1)],
                                    start=(dc == 0), stop=(dc == NDC - 1))
                            nc.vector.scalar_tensor_tensor(
                                out=V_sb[:, ck, 8 * half:8 * (half + 1), 0:DH],
                                in0=vps[:].rearrange("p (h d) -> p h d", h=8),
                                scalar=1.0,
                                in1=bv_sb[:, 512 * half:512 * (half + 1)].rearrange(
                                    "p (h d) -> p h d", h=8),
                                op0=Alu.mult, op1=Alu.add)
                        nc.vector.memset(V_sb[:, ck, :, DH:DH + 1], 1.0)

                # ---------- phases B2+C: per pair K,Q then attention -------
                with tc.tile_pool(name=f"kq{rl}", bufs=2) as kqp, \
                     tc.tile_pool(name=f"pt{rl}", bufs=3) as pp, \
                     tc.tile_pool(name=f"psC{rl}", bufs=1, space="PSUM") as psC:
                    for p in range(NDC):
                        kt = kqp.tile([128, T], BF16, name="kt")
                        for ts in range(4):
                            kps = psC.tile([128, 512], F32, name="kqps", bufs=2)
                            for dc in range(NDC):
                                nc.tensor.matmul(
                                    kps[:],
                                    lhsT=wqk_sb[:, dc, D + 128 * p:D + 128 * (p + 1)],
                                    rhs=xnT[:, dc, 512 * ts:512 * (ts + 1)],
                                    start=(dc == 0), stop=(dc == NDC - 1))
                            nc.scalar.add(out=kt[:, 512 * ts:512 * (ts + 1)],
                                          in_=kps[:], add=bqk_sb[:, 8 + p:9 + p])
                        qt = kqp.tile([128, 1024], BF16, name="qt")
                        for ts in range(2):
                            qps = psC.tile([128, 512], F32, name="kqps", bufs=2)
                            for dc in range(NDC):
                                nc.tensor.matmul(
                                    qps[:],
                                    lhsT=wqk_sb[:, dc, 128 * p:128 * (p + 1)],
                                    rhs=xnT[:, dc, 512 * ts:512 * (ts + 1)],
                                    start=(dc == 0), stop=(dc == NDC - 1))
                            nc.scalar.add(out=qt[:, 512 * ts:512 * (ts + 1)],
                                          in_=qps[:], add=bqk_sb[:, p:p + 1])

                        for qsb in range(2):
                            E = E_LO if qsb == 0 else E_HI
                            base = 0 if qsb == 0 else E_LO
                            qoff = 512 * qsb
                            o_psA = psC.tile([65, 512], F32, name="o_psA")
                            o_psB = psC.tile([65, 512], F32, name="o_psB")
                            o_ps = [o_psA, o_psB]
                            for u in range(E):
                                sps = psC.tile([128, 1024], F32, name="sps",
                                               bufs=2)
                                for j in range(2):  # head A | head B
                                    nc.tensor.matmul(
                                        sps[:, 512 * j:512 * (j + 1)],
                                        lhsT=kt[64 * j:64 * (j + 1),
                                                128 * u:128 * (u + 1)],
                                        rhs=qt[64 * j:64 * (j + 1),
                                               qoff:qoff + 512],
                                        start=True, stop=True,
                                        tile_position=(64 * j, 0))
                                P = pp.tile([128, 1024], BF16, name="P")
                                nc.scalar.activation(
                                    out=P[:], in_=sps[:], func=Exp,
                                    bias=sbias_sb[:, base + u:base + u + 1],
                                    scale=0.125)
                                dslot = u if qsb == 0 else u - 4
                                if 0 <= dslot < 4:
                                    for j in range(2):
                                        nc.vector.tensor_mul(
                                            out=P[:, 512 * j:512 * (j + 1)],
                                            in0=P[:, 512 * j:512 * (j + 1)],
                                            in1=dmask_sb[:, dslot, :])
                                for j in range(2):
                                    nc.tensor.matmul(
                                        o_ps[j][:],
                                        lhsT=V_sb[:, u, 2 * p + j, :],
                                        rhs=P[:, 512 * j:512 * (j + 1)],
                                        start=(u == 0), stop=(u == E - 1))
                            for j in range(2):
                                recip = pp.tile([1, 512], F32, name="recip")
                                nc.vector.reciprocal(out=recip[:],
                                                     in_=o_ps[j][64:65, :])
                                rb = psC.tile([64, 512], F32, name="kqps",
                                              bufs=2)
                                nc.tensor.matmul(rb[:], lhsT=ones64_sb[:],
                                                 rhs=recip[:], start=True,
                                                 stop=True)
                                oU = pp.tile([64, 512], BF16, name="oU")
                                nc.vector.tensor_copy(out=oU[:],
                                                      in_=o_ps[j][0:64, :])
                                nc.vector.tensor_mul(
                                    out=oT[64 * j:64 * (j + 1), p,
                                           qoff:qoff + 512],
                                    in0=oU[:], in1=rb[:])

                # ---------- phase D: out projection + residual ----------
                with tc.tile_pool(name=f"oD{rl}", bufs=2) as dpool, \
                     tc.tile_pool(name=f"woD{rl}", bufs=1) as wop, \
                     tc.tile_pool(name=f"psD{rl}", bufs=1, space="PSUM") as psD:
                    wo_sb = wop.tile([128, NDC, D], BF16, name="wo_sb")
                    for dc in range(NDC):
                        nc.sync.dma_start(out=wo_sb[:, dc, :],
                                          in_=wo[li, 128 * dc:128 * (dc + 1), :])
                    for qb in range(8):
                        h_tile = dpool.tile([128, D], F32, name="h_res")
                        if li == 0:
                            nc.sync.dma_start(
                                out=h_tile[:],
                                in_=h0[128 * qb:128 * (qb + 1), :])
                        else:
                            nc.sync.dma_start(
                                out=h_tile[:],
                                in_=agin1[128 * qb:128 * (qb + 1), :])
                        hn = dpool.tile([128, D], F32, name="hn")
                        for half in range(2):
                            ops = psD.tile([128, 512], F32, name="ops", bufs=2)
                            for dc in range(NDC):
                                nc.tensor.matmul(
                                    ops[:],
                                    lhsT=oT[:, dc, 128 * qb:128 * (qb + 1)],
                                    rhs=wo_sb[:, dc, 512 * half:512 * (half + 1)],
                                    start=(dc == 0), stop=(dc == NDC - 1))
                            nc.vector.scalar_tensor_tensor(
                                out=hn[:, 512 * half:512 * (half + 1)],
                                in0=ops[:], scalar=1.0,
                                in1=h_tile[:, 512 * half:512 * (half + 1)],
                                op0=Alu.mult, op1=Alu.add)
                            nc.vector.tensor_add(
                                out=hn[:, 512 * half:512 * (half + 1)],
                                in0=hn[:, 512 * half:512 * (half + 1)],
                                in1=bo_sb[:, 512 * half:512 * (half + 1)])
                        if li == 0:
                            nc.sync.dma_start(
                                out=agin1[128 * qb:128 * (qb + 1), :],
                                in_=hn[:])
                        elif qb == 7:
                            nc.sync.dma_start(out=agin2[0:1, :],
                                              in_=hn[127:128, :])

            if li == 0:
                nc.gpsimd.collective_compute(
                    "AllGather", Alu.bypass,
                    replica_groups=[list(range(NCORES))],
                    ins=[agin1.opt()], outs=[agout1.opt()])
                if DEBUG:
                    nc.sync.dma_start(out=dbg_h1own[:], in_=agin1[:])
            else:
                nc.gpsimd.collective_compute(
                    "AllGather", Alu.bypass,
                    replica_groups=[list(range(NCORES))],
                    ins=[agin2.opt()], outs=[agout2.opt()])

        # ---------- head: final LN + vocab-sharded projection ----------
        with tc.tile_pool(name="hd", bufs=1) as hd, \
             tc.tile_pool(name="hdw", bufs=2) as hdw:
            x4 = hd.tile([4, D], F32)
            agout2_last = agout2s[repeats - 1]
            for i in range(4):
                nc.sync.dma_start(out=x4[i:i + 1, :],
                                  in_=agout2_last[2 * i:2 * i + 1, :])
            if DEBUG:
                nc.sync.dma_start(out=dbg_x4[:], in_=x4[:])
            xnf = hd.tile([4, D], F32)
            layernorm_tile(hd, x4, xnf, p=4)
            lnfg_sb = hd.tile([4, D], F32)
            nc.sync.dma_start(out=lnfg_sb[:], in_=lnfg[:])
            lnfb_sb = hd.tile([4, D], F32)
            nc.sync.dma_start(out=lnfb_sb[:], in_=lnfb[:])
            nc.vector.tensor_mul(out=xnf[:], in0=xnf[:], in1=lnfg_sb[:])
            nc.vector.tensor_add(out=xnf[:], in0=xnf[:], in1=lnfb_sb[:])

            ident_sb = hd.tile([128, 128], F32)
            nc.sync.dma_start(out=ident_sb[:], in_=ident[:])
            xhT = hd.tile([128, NDC, 4], BF16)
            with tc.tile_pool(name="psT", bufs=1, space="PSUM") as psT:
                for dc in range(NDC):
                    tps = psT.tile([128, 4], F32, name="tps", bufs=2)
                    nc.tensor.transpose(out=tps[:],
                                        in_=xnf[:, 128 * dc:128 * (dc + 1)],
                                        identity=ident_sb[0:4, 0:4])
                    nc.vector.tensor_copy(out=xhT[:, dc, :], in_=tps[:])

            headb_sb = hd.tile([4, CSH], F32)
            nc.sync.dma_start(out=headb_sb[:], in_=headb[:])
            with tc.tile_pool(name="psL", bufs=1, space="PSUM") as psL:
                lps = [psL.tile([4, 500], F32, name=f"lps{nb}")
                       for nb in range(8)]
                for dc in range(NDC):
                    hw = hdw.tile([128, CSH], BF16, name="hw")
                    nc.sync.dma_start(out=hw[:],
                                      in_=headw[128 * dc:128 * (dc + 1), :])
                    for nb in range(8):
                        nc.tensor.matmul(lps[nb][:], lhsT=xhT[:, dc, :],
                                         rhs=hw[:, 500 * nb:500 * (nb + 1)],
                                         start=(dc == 0), stop=(dc == NDC - 1))
                for nb in range(8):
                    lsb = hdw.tile([4, 500], F32, name="lsb")
                    nc.vector.tensor_add(out=lsb[:], in0=lps[nb][:],
                                         in1=headb_sb[:, 500 * nb:500 * (nb + 1)])
                    nc.sync.dma_start(out=logits[:, 500 * nb:500 * (nb + 1)],
                                      in_=lsb[:])

    nc.compile()
    return nc


def get_nc(repeats=1):
    key = f"nc{repeats}"
    if key not in _CACHE:
        _CACHE[key] = _build(repeats)
    return _CACHE[key]


def make_runner(in_maps, repeats=1):
    """Returns run_once() -> (out_arrs, wall_seconds) with device-cached
    inputs and a pre-traced executable (mirrors bass2jax.run_bass_via_pjrt)."""
    import time as _time
    import jax
    from jax.sharding import Mesh, PartitionSpec, NamedSharding
    from jax.experimental.shard_map import shard_map
    from concourse import bass2jax

    nc = get_nc(repeats)
    bass2jax.install_neuronx_cc_hook()
    partition_name = (nc.partition_id_tensor.name
                      if nc.partition_id_tensor else None)
    in_names, out_names, out_avals, zero_outs = [], [], [], []
    for alloc in nc.m.functions[0].allocations:
        if not isinstance(alloc, mybir.MemoryLocationSet):
            continue
        name = alloc.memorylocations[0].name
        if alloc.kind == "ExternalInput":
            if name != partition_name:
                in_names.append(name)
        elif alloc.kind == "ExternalOutput":
            shape = tuple(alloc.tensor_shape)
            dtype = mybir.dt.np(alloc.dtype)
            out_names.append(name)
            out_avals.append(jax.core.ShapedArray(shape, dtype))
            zero_outs.append(np.zeros(shape, dtype))
    n_params, n_outs = len(in_names), len(out_names)
    all_in = list(in_names) + list(out_names)
    if partition_name:
        all_in.append(partition_name)

    def _body(*args):
        operands = list(args)
        if partition_name:
            operands.append(bass2jax.partition_id_tensor())
        outs = bass2jax._bass_exec_p.bind(
            *operands, out_avals=tuple(out_avals), in_names=tuple(all_in),
            out_names=tuple(out_names), lowering_input_output_aliases=(),
            sim_require_finite=True, sim_require_nnan=True, nc=nc)
        return tuple(outs)

    devices = jax.devices()[:NCORES]
    mesh = Mesh(np.asarray(devices), ("core",))
    in_specs = (PartitionSpec("core"),) * (n_params + n_outs)
    out_specs = (PartitionSpec("core"),) * n_outs
    donate = tuple(range(n_params, n_params + n_outs))
    sharded = jax.jit(shard_map(_body, mesh=mesh, in_specs=in_specs,
                                out_specs=out_specs, check_rep=False),
                      donate_argnums=donate, keep_unused=True)
    sh = NamedSharding(mesh, PartitionSpec("core"))
    dev_in = [jax.device_put(
        np.concatenate([np.asarray(in_maps[c][k]) for c in range(NCORES)], 0), sh)
        for k in in_names]

    def run_once():
        dz = [jax.device_put(
            np.zeros((NCORES * z.shape[0], *z.shape[1:]), z.dtype), sh)
            for z in zero_outs]
        t0 = _time.time()
        out = sharded(*dev_in, *dz)
        jax.block_until_ready(out)
        dt = _time.time() - t0
        return dict(zip(out_names, out)), dt

    return run_once


def run_spmd(in_maps):
    nc = get_nc()
    return bass_utils.run_bass_kernel_spmd(nc, in_maps, core_ids=list(range(NCORES)))


def kernel(**inputs) -> np.ndarray:
    in_maps = _host_prep(inputs)
    res = run_spmd(in_maps)
    out = np.empty((B, C), np.float32)
    for c in range(NCORES):
        out[:, CSH * c:CSH * (c + 1)] = res.results[c]["logits"]
    _CACHE["last_results"] = res
    return out


# revision 19
# speedup vs baseline: 39807.1242x; 2.5957x over previous
"""Trainium2 Bass kernel for nn_BubbleTransformer (2-layer attention-only
transformer, B=4 T=2048 D=1024 H=16, vocab 32000, logits of last token).

Distribution over 8 NeuronCores (one chip, LNC1):
  core c = 2*b + s  handles batch b = c//2 and query-half s = c%2.
  Tokens are PERMUTED per core so that the core's own queries always sit at
  positions 0-1023 (uniform SPMD instruction stream; all per-core variation
  is carried by input data: gather indices, positional rows, mask bias).
  K/V are computed for all 2048 tokens (replicated within the batch pair);
  Q / attention / out-projection only for the core's own 1024 queries.
  After layer 1 an 8-rank AllGather exchanges the per-core residual rows;
  after layer 2 a tiny AllGather shares the last-token rows; the vocab
  projection is sharded 8 x 4000 columns.

Compute is bf16 on the TensorEngine with fp32 PSUM accumulation; softmax is
exp on ScalarE with the causal mask folded into a per-slot exp bias
(0 or -30000) plus a small constant diagonal mask multiply on DVE.
"""

import os
from contextlib import ExitStack

import numpy as np
import ml_dtypes

import concourse.bass as bass
import concourse.tile as tile
from concourse import bacc, mybir
from concourse import bass_utils

F32 = mybir.dt.float32
BF16 = mybir.dt.bfloat16
I32 = mybir.dt.int32

V = 32000
D = 1024
H = 16
L = 2
T = 2048
B = 4
C = 32000
DH = 64
EPS = 1e-5

NCORES = 8
CSH = C // NCORES          # 4000 vocab columns per core
NCK = T // 128             # 16 token chunks
NDC = D // 128             # 8 d-chunks
E_LO, E_HI = 12, 16        # padded causal extents (in 128-key blocks)
NSLOT = E_LO + E_HI        # 28 schedule slots
NEG = -30000.0             # exp bias for fully-masked slots

# own-first permutation of the four 512-token superblocks
QSB_ORDER = {0: [0, 3, 1, 2], 1: [1, 2, 0, 3]}

DEBUG = bool(int(os.environ.get("BT_DEBUG", "0")))

_CACHE = {}


def _bf16(x):
    return np.asarray(x, np.float32).astype(ml_dtypes.bfloat16)


def _gathered_row(t, b):
    """Row of the global token (b, t) inside the layer-1 AllGather output."""
    q, r = t // 512, t % 512
    off = {0: 0, 3: 512, 1: 1024, 2: 1536}[q]
    return 2048 * b + off + r


def _host_prep(inputs):
    """Builds the per-core input maps (list of dict name->np array)."""
    tokens = np.asarray(inputs["tokens"]).astype(np.int32)      # [B, T]
    embw = _bf16(inputs["embed_W"])                             # [V, D]
    posW = np.asarray(inputs["pos_W"], np.float32)              # [T, D]
    ln_g = np.asarray(inputs["ln_g"], np.float32)               # [L, D]
    ln_b = np.asarray(inputs["ln_b"], np.float32)
    qkv_W = np.asarray(inputs["qkv_W"], np.float32)             # [L, D, 3D]
    qkv_b = np.asarray(inputs["qkv_b"], np.float32)             # [L, 3D]
    out_W = np.asarray(inputs["out_W"], np.float32)             # [L, D, D]
    out_b = np.asarray(inputs["out_b"], np.float32)             # [L, D]
    lnf_g = np.asarray(inputs["lnf_g"], np.float32)
    lnf_b = np.asarray(inputs["lnf_b"], np.float32)
    head_W = np.asarray(inputs["head_W"], np.float32)           # [D, C]
    head_b = np.asarray(inputs["head_b"], np.float32)           # [C]

    # fold the pre-attention LN affine into the qkv projection
    wqk = np.empty((L, D, 2 * D), ml_dtypes.bfloat16)
    wv = np.empty((L, D, D), ml_dtypes.bfloat16)
    bqkv = np.empty((L, 3 * D), np.float32)
    for l in range(L):
        weff = qkv_W[l] * ln_g[l][:, None]
        wqk[l] = _bf16(weff[:, :2 * D])
        wv[l] = _bf16(weff[:, 2 * D:])
        bqkv[l] = qkv_b[l] + ln_b[l] @ qkv_W[l]

    # Q/K biases as per-partition columns: bqk[l, p, j] ; j<8 -> Q chunk j,
    # j>=8 -> K chunk j-8
    bqk = np.empty((L, 128, 16), np.float32)
    for l in range(L):
        for j in range(8):
            bqk[l, :, j] = bqkv[l, 128 * j:128 * (j + 1)]
            bqk[l, :, 8 + j] = bqkv[l, D + 128 * j:D + 128 * (j + 1)]
    bv = np.tile(bqkv[:, None, 2 * D:], (1, 128, 1)).astype(np.float32)   # [L,128,D]
    bo = np.tile(out_b[:, None, :], (1, 128, 1)).astype(np.float32)       # [L,128,D]

    wo = _bf16(out_W)                                                     # [L, D, D]

    # diagonal causal masks, [128(k), 4(j), 512(q)]: 1 where q >= 128*j + k
    kk = np.arange(128)[:, None]
    qq = np.arange(512)[None, :]
    dmask = np.stack([(qq >= 128 * j + kk) for j in range(4)], axis=1)
    dmask = dmask.astype(ml_dtypes.bfloat16)                              # [128,4,512]

    ident = np.eye(128, dtype=np.float32)
    ones64 = np.ones((1, 64), np.float32)
    lnfg_t = np.tile(lnf_g[None, :], (4, 1)).astype(np.float32)
    lnfb_t = np.tile(lnf_b[None, :], (4, 1)).astype(np.float32)

    in_maps = []
    for c in range(NCORES):
        b, s = divmod(c, 2)
        order = QSB_ORDER[s]
        perm = np.concatenate([np.arange(512) + 512 * q for q in order])  # [T]

        tokidx = tokens[b][perm].reshape(NCK, 128).T.copy()               # [128,16]
        gidx = np.array([_gathered_row(t, b) for t in perm], np.int32)
        gidx = gidx.reshape(NCK, 128).T.copy()                            # [128,16]
        posw_c = posW[perm].copy()                                        # [T, D]

        # per-slot exp bias: 0 for diag/valid slots, NEG for future-key slots
        sb = np.zeros(NSLOT, np.float32)
        for u in range(E_LO):                                             # lo: q = order[0]
            if u >= 4 and order[u // 4] > order[0]:
                sb[u] = NEG
        for u in range(E_HI):                                             # hi: q = order[1]
            if (u // 4) != 1 and order[u // 4] > order[1]:
                sb[E_LO + u] = NEG
        sbias = np.tile(sb[None, :], (128, 1)).astype(np.float32)         # [128,28]

        # layer-2 (decode) per-4-slot-group exp bias: queries are the last
        # 128 own positions (pos 896-1023); key group g = order[g]
        sb2 = np.zeros(4, np.float32)
        for g in range(4):
            if g != 1 and order[g] > order[1]:
                sb2[g] = NEG
        sbias2 = np.tile(sb2[None, :], (128, 1)).astype(np.float32)       # [128,4]

        headw_c = _bf16(head_W[:, CSH * c:CSH * (c + 1)])                 # [D, 4000]
        headb_c = np.tile(head_b[None, CSH * c:CSH * (c + 1)], (4, 1)).astype(np.float32)

        in_maps.append({
            "tokidx": tokidx, "gidx": gidx, "posw": posw_c,
            "embw": embw, "wqk": wqk, "wv": wv, "bqk": bqk, "bv": bv,
            "wo": wo, "bo": bo, "dmask": dmask, "sbias": sbias,
            "sbias2": sbias2,
            "headw": headw_c, "headb": headb_c,
            "lnfg": lnfg_t, "lnfb": lnfb_t,
            "ident": ident, "ones64": ones64,
        })
    return in_maps


def _build(repeats=1):
    nc = bacc.Bacc("TRN2", target_bir_lowering=False, debug=False,
                   num_devices=NCORES)

    def din(name, shape, d):
        return nc.dram_tensor(name, shape, d, kind="ExternalInput").ap()

    tokidx = din("tokidx", [128, NCK], I32)
    gidx = din("gidx", [128, NCK], I32)
    posw = din("posw", [T, D], F32)
    embw = din("embw", [V, D], BF16)
    wqk = din("wqk", [L, D, 2 * D], BF16)
    wv = din("wv", [L, D, D], BF16)
    bqk = din("bqk", [L, 128, 16], F32)
    bv = din("bv", [L, 128, D], F32)
    wo = din("wo", [L, D, D], BF16)
    bo = din("bo", [L, 128, D], F32)
    dmask = din("dmask", [128, 4, 512], BF16)
    sbias = din("sbias", [128, NSLOT], F32)
    sbias2 = din("sbias2", [128, 4], F32)
    headw = din("headw", [D, CSH], BF16)
    headb = din("headb", [4, CSH], F32)
    lnfg = din("lnfg", [4, D], F32)
    lnfb = din("lnfb", [4, D], F32)
    ident = din("ident", [128, 128], F32)
    ones64 = din("ones64", [1, 64], F32)

    logits = nc.dram_tensor("logits", [4, CSH], F32, kind="ExternalOutput").ap()
    if DEBUG:
        dbg_h0 = nc.dram_tensor("dbg_h0", [T, D], F32, kind="ExternalOutput").ap()
        dbg_h1own = nc.dram_tensor("dbg_h1own", [1024, D], F32,
                                   kind="ExternalOutput").ap()
        dbg_x4 = nc.dram_tensor("dbg_x4", [4, D], F32, kind="ExternalOutput").ap()

    Exp = mybir.ActivationFunctionType.Exp
    Sqrt = mybir.ActivationFunctionType.Sqrt
    Alu = mybir.AluOpType

    with tile.TileContext(nc) as tc, ExitStack() as ctx:
        dram = ctx.enter_context(tc.tile_pool(name="dram", bufs=1, space="DRAM"))
        h0 = dram.tile([T, D], F32)
        agin1s, agout1s, agin2s, agout2s = [], [], [], []
        for r in range(repeats):
            agin1s.append(dram.tile([1024, D], F32, name=f"agin1_{r}"))
            agout1s.append(dram.tile([NCORES * 1024, D], F32,
                                     addr_space="Shared", name=f"agout1_{r}"))
            agin2s.append(dram.tile([1, D], F32, name=f"agin2_{r}"))
            agout2s.append(dram.tile([NCORES, D], F32,
                                     addr_space="Shared", name=f"agout2_{r}"))

        consts = ctx.enter_context(tc.tile_pool(name="consts", bufs=1))

        tok_sb = consts.tile([128, NCK], I32)
        nc.sync.dma_start(out=tok_sb[:], in_=tokidx[:])
        gidx_sb = consts.tile([128, NCK], I32)
        nc.sync.dma_start(out=gidx_sb[:], in_=gidx[:])
        dmask_sb = consts.tile([128, 4, 512], BF16)
        nc.sync.dma_start(out=dmask_sb[:], in_=dmask[:])
        sbias_sb = consts.tile([128, NSLOT], F32)
        nc.sync.dma_start(out=sbias_sb[:], in_=sbias[:])
        sbias2_sb = consts.tile([128, 4], F32)
        nc.sync.dma_start(out=sbias2_sb[:], in_=sbias2[:])
        ones64_sb = consts.tile([1, 64], F32)
        nc.sync.dma_start(out=ones64_sb[:], in_=ones64[:])
        eps_sb = consts.tile([128, 1], F32)
        nc.vector.memset(eps_sb[:], EPS)

        def layernorm_tile(pool, h_tile, xn_tile, p=128):
            """xn = (h - mean) * rsqrt(var + eps), fp32 -> bf16, [p, D]."""
            stats = pool.tile([128, 2, 6], F32, name="stats")
            nc.vector.bn_stats(out=stats[:p, 0, :], in_=h_tile[:p, 0:512])
            nc.vector.bn_stats(out=stats[:p, 1, :], in_=h_tile[:p, 512:1024])
            mv = pool.tile([128, 2], F32, name="mv")
            nc.vector.bn_aggr(out=mv[:p], in_=stats[:p])
            rstd = pool.tile([128, 1], F32, name="rstd")
            nc.scalar.activation(out=rstd[:p], in_=mv[:p, 1:2], func=Sqrt,
                                 bias=eps_sb[:p], scale=1.0)
            nc.vector.reciprocal(out=rstd[:p], in_=rstd[:p])
            nc.vector.tensor_scalar(out=xn_tile[:p], in0=h_tile[:p],
                                    scalar1=mv[:p, 0:1], scalar2=rstd[:p],
                                    op0=Alu.subtract, op1=Alu.mult)

        for rep, li in [(r, l) for r in range(repeats) for l in range(L)]:
            rl = f"{rep}_{li}"
            agin1, agout1 = agin1s[rep], agout1s[rep]
            agin2, agout2 = agin2s[rep], agout2s[rep]
            lyr = ExitStack()
            with lyr:
                lw = lyr.enter_context(
                    tc.tile_pool(name=f"lw{rl}", bufs=1))
                wqk_sb = lw.tile([128, NDC, 2 * D], BF16, name="wqk_sb")
                for dc in range(NDC):
                    nc.sync.dma_start(out=wqk_sb[:, dc, :],
                                      in_=wqk[li, 128 * dc:128 * (dc + 1), :])
                bqk_sb = lw.tile([128, 16], F32, name="bqk_sb")
                nc.sync.dma_start(out=bqk_sb[:], in_=bqk[li])
                bv_sb = lw.tile([128, D], F32, name="bv_sb")
                nc.sync.dma_start(out=bv_sb[:], in_=bv[li])
                bo_sb = lw.tile([128, D], F32, name="bo_sb")
                nc.sync.dma_start(out=bo_sb[:], in_=bo[li])

                xnT = lw.tile([128, NDC, T], BF16, name="xnT")
                V_sb = lw.tile([128, NCK, H, DH + 1], BF16, name="V_sb")
                oT = lw.tile([128, NDC, 1024], BF16, name="oT")

                # ---------- phase A: residual source + LN + transpose ------
                with tc.tile_pool(name=f"lnA{rl}", bufs=2) as lnp:
                    for ck in range(NCK):
                        h_tile = lnp.tile([128, D], F32, name="h_tile")
                        if li == 0:
                            emb = lnp.tile([128, D], BF16, name="emb")
                            nc.gpsimd.indirect_dma_start(
                                out=emb[:], out_offset=None, in_=embw[:],
                                in_offset=bass.IndirectOffsetOnAxis(
                                    ap=tok_sb[:, ck:ck + 1], axis=0))
                            pos = lnp.tile([128, D], F32, name="pos")
                            nc.sync.dma_start(
                                out=pos[:], in_=posw[128 * ck:128 * (ck + 1), :])
                            nc.vector.tensor_add(out=h_tile[:], in0=emb[:],
                                                 in1=pos[:])
                            nc.sync.dma_start(
                                out=h0[128 * ck:128 * (ck + 1), :], in_=h_tile[:])
                            if DEBUG:
                                nc.sync.dma_start(
                                    out=dbg_h0[128 * ck:128 * (ck + 1), :],
                                    in_=h_tile[:])
                        elif ck < 8:
                            nc.sync.dma_start(
                                out=h_tile[:],
                                in_=agin1[128 * ck:128 * (ck + 1), :])
                        else:
                            nc.gpsimd.indirect_dma_start(
                                out=h_tile[:], out_offset=None, in_=agout1[:],
                                in_offset=bass.IndirectOffsetOnAxis(
                                    ap=gidx_sb[:, ck:ck + 1], axis=0))
                        xn = lnp.tile([128, D], BF16, name="xn")
                        layernorm_tile(lnp, h_tile, xn)
                        for dc in range(NDC):
                            nc.sync.dma_start_transpose(
                                out=xnT[:, dc, 128 * ck:128 * (ck + 1)],
                                in_=xn[:, 128 * dc:128 * (dc + 1)])

                # ---------- phase B1: V for all tokens ----------
                with tc.tile_pool(name=f"wv{rl}", bufs=1) as wvp, \
                     tc.tile_pool(name=f"psB{rl}", bufs=1, space="PSUM") as psB:
                    wv_sb = wvp.tile([128, NDC, D], BF16, name="wv_sb")
                    for dc in range(NDC):
                        nc.sync.dma_start(out=wv_sb[:, dc, :],
                                          in_=wv[li, 128 * dc:128 * (dc + 1), :])
                    for ck in range(NCK):
                        for half in range(2):
                            vps = psB.tile([128, 512], F32, name="vps", bufs=2)
                            for dc in range(NDC):
                                nc.tensor.matmul(
                                    vps[:],
                                    lhsT=xnT[:, dc, 128 * ck:128 * (ck + 1)],
                                    rhs=wv_sb[:, dc, 512 * half:512 * (half + 1)],
                                    start=(dc == 0), stop=(dc == NDC - 1))
                            nc.vector.scalar_tensor_tensor(
                                out=V_sb[:, ck, 8 * half:8 * (half + 1), 0:DH],
                                in0=vps[:].rearrange("p (h d) -> p h d", h=8),
                                scalar=1.0,
                                in1=bv_sb[:, 512 * half:512 * (half + 1)].rearrange(
                                    "p (h d) -> p h d", h=8),
                                op0=Alu.mult, op1=Alu.add)
                        nc.vector.memset(V_sb[:, ck, :, DH:DH + 1], 1.0)

                # ---------- phases B2+C: per pair K,Q then attention -------
                with tc.tile_pool(name=f"kq{rl}", bufs=2) as kqp, \
                     tc.tile_pool(name=f"pt{rl}", bufs=3) as pp, \
                     tc.tile_pool(name=f"psC{rl}", bufs=1, space="PSUM") as psC:
                  if li == L - 1:
                    # decode layer: only the last 128 own queries matter
                    for p in range(NDC):
                        kt = kqp.tile([128, T], BF16, name="kt")
                        for ts in range(4):
                            kps = psC.tile([128, 512], F32, name="kqps", bufs=2)
                            for dc in range(NDC):
                                nc.tensor.matmul(
                                    kps[:],
                                    lhsT=wqk_sb[:, dc, D + 128 * p:D + 128 * (p + 1)],
                                    rhs=xnT[:, dc, 512 * ts:512 * (ts + 1)],
                                    start=(dc == 0), stop=(dc == NDC - 1))
                            nc.scalar.add(out=kt[:, 512 * ts:512 * (ts + 1)],
                                          in_=kps[:], add=bqk_sb[:, 8 + p:9 + p])
                        qt = kqp.tile([128, 128], BF16, name="qt")
                        qps = psC.tile([128, 128], F32, name="kqps", bufs=2)
                        for dc in range(NDC):
                            nc.tensor.matmul(
                                qps[:],
                                lhsT=wqk_sb[:, dc, 128 * p:128 * (p + 1)],
                                rhs=xnT[:, dc, 896:1024],
                                start=(dc == 0), stop=(dc == NDC - 1))
                        nc.scalar.add(out=qt[:], in_=qps[:],
                                      add=bqk_sb[:, p:p + 1])

                        o_psA = psC.tile([65, 128], F32, name="o_psA")
                        o_psB = psC.tile([65, 128], F32, name="o_psB")
                        o_ps = [o_psA, o_psB]
                        for g in range(4):
                            sps = psC.tile([128, 1024], F32, name="sps", bufs=2)
                            for t in range(4):
                                u = 4 * g + t
                                for j in range(2):
                                    nc.tensor.matmul(
                                        sps[:, 512 * j + 128 * t:
                                            512 * j + 128 * (t + 1)],
                                        lhsT=kt[64 * j:64 * (j + 1),
                                                128 * u:128 * (u + 1)],
                                        rhs=qt[64 * j:64 * (j + 1), :],
                                        start=True, stop=True,
                                        tile_position=(64 * j, 0))
                            P = pp.tile([128, 1024], BF16, name="P")
                            nc.scalar.activation(
                                out=P[:], in_=sps[:], func=Exp,
                                bias=sbias2_sb[:, g:g + 1], scale=0.125)
                            if g == 1:  # slot 7 is the diagonal block
                                for j in range(2):
                                    nc.vector.tensor_mul(
                                        out=P[:, 512 * j + 384:512 * j + 512],
                                        in0=P[:, 512 * j + 384:512 * j + 512],
                                        in1=dmask_sb[:, 0, 0:128])
                            for t in range(4):
                                u = 4 * g + t
                                for j in range(2):
                                    nc.tensor.matmul(
                                        o_ps[j][:],
                                        lhsT=V_sb[:, u, 2 * p + j, :],
                                        rhs=P[:, 512 * j + 128 * t:
                                              512 * j + 128 * (t + 1)],
                                        start=(u == 0), stop=(u == 15))
                        for j in range(2):
                            recip = pp.tile([1, 128], F32, name="recip")
                            nc.vector.reciprocal(out=recip[:],
                                                 in_=o_ps[j][64:65, :])
                            rb = psC.tile([64, 128], F32, name="kqps", bufs=2)
                            nc.tensor.matmul(rb[:], lhsT=ones64_sb[:],
                                             rhs=recip[:], start=True,
                                             stop=True)
                            oU = pp.tile([64, 128], BF16, name="oU")
                            nc.vector.tensor_copy(out=oU[:],
                                                  in_=o_ps[j][0:64, :])
                            nc.vector.tensor_mul(
                                out=oT[64 * j:64 * (j + 1), p, 0:128],
                                in0=oU[:], in1=rb[:])
                  else:
                    for p in range(NDC):
                        kt = kqp.tile([128, T], BF16, name="kt")
                        for ts in range(4):
                            kps = psC.tile([128, 512], F32, name="kqps", bufs=2)
                            for dc in range(NDC):
                                nc.tensor.matmul(
                                    kps[:],
                                    lhsT=wqk_sb[:, dc, D + 128 * p:D + 128 * (p + 1)],
                                    rhs=xnT[:, dc, 512 * ts:512 * (ts + 1)],
                                    start=(dc == 0), stop=(dc == NDC - 1))
                            nc.scalar.add(out=kt[:, 512 * ts:512 * (ts + 1)],
                                          in_=kps[:], add=bqk_sb[:, 8 + p:9 + p])
                        qt = kqp.tile([128, 1024], BF16, name="qt")
                        for ts in range(2):
                            qps = psC.tile([128, 512], F32, name="kqps", bufs=2)
                            for dc in range(NDC):
                                nc.tensor.matmul(
                                    qps[:],
                                    lhsT=wqk_sb[:, dc, 128 * p:128 * (p + 1)],
                                    rhs=xnT[:, dc, 512 * ts:512 * (ts + 1)],
                                    start=(dc == 0), stop=(dc == NDC - 1))
                            nc.scalar.add(out=qt[:, 512 * ts:512 * (ts + 1)],
                                          in_=qps[:], add=bqk_sb[:, p:p + 1])

                        for qsb in range(2):
                            E = E_LO if qsb == 0 else E_HI
                            base = 0 if qsb == 0 else E_LO
                            qoff = 512 * qsb
                            o_psA = psC.tile([65, 512], F32, name="o_psA")
                            o_psB = psC.tile([65, 512], F32, name="o_psB")
                            o_ps = [o_psA, o_psB]
                            for u in range(E):
                                sps = psC.tile([128, 1024], F32, name="sps",
                                               bufs=2)
                                for j in range(2):  # head A | head B
                                    nc.tensor.matmul(
                                        sps[:, 512 * j:512 * (j + 1)],
                                        lhsT=kt[64 * j:64 * (j + 1),
                                                128 * u:128 * (u + 1)],
                                        rhs=qt[64 * j:64 * (j + 1),
                                               qoff:qoff + 512],
                                        start=True, stop=True,
                                        tile_position=(64 * j, 0))
                                P = pp.tile([128, 1024], BF16, name="P")
                                nc.scalar.activation(
                                    out=P[:], in_=sps[:], func=Exp,
                                    bias=sbias_sb[:, base + u:base + u + 1],
                                    scale=0.125)
                                dslot = u if qsb == 0 else u - 4
                                if 0 <= dslot < 4:
                                    for j in range(2):
                                        nc.vector.tensor_mul(
                                            out=P[:, 512 * j:512 * (j + 1)],
                                            in0=P[:, 512 * j:512 * (j + 1)],
                                            in1=dmask_sb[:, dslot, :])
                                for j in range(2):
                                    nc.tensor.matmul(
                                        o_ps[j][:],
                                        lhsT=V_sb[:, u, 2 * p + j, :],
                                        rhs=P[:, 512 * j:512 * (j + 1)],
                                        start=(u == 0), stop=(u == E - 1))
                            for j in range(2):
                                recip = pp.tile([1, 512], F32, name="recip")
                                nc.vector.reciprocal(out=recip[:],
                                                     in_=o_ps[j][64:65, :])
                                rb = psC.tile([64, 512], F32, name="kqps",
                                              bufs=2)
                                nc.tensor.matmul(rb[:], lhsT=ones64_sb[:],
                                                 rhs=recip[:], start=True,
                                                 stop=True)
                                oU = pp.tile([64, 512], BF16, name="oU")
                                nc.vector.tensor_copy(out=oU[:],
                                                      in_=o_ps[j][0:64, :])
                                nc.vector.tensor_mul(
                                    out=oT[64 * j:64 * (j + 1), p,
                                           qoff:qoff + 512],
                                    in0=oU[:], in1=rb[:])

                # ---------- phase D: out projection + residual ----------
                with tc.tile_pool(name=f"oD{rl}", bufs=2) as dpool, \
                     tc.tile_pool(name=f"woD{rl}", bufs=1) as wop, \
                     tc.tile_pool(name=f"psD{rl}", bufs=1, space="PSUM") as psD:
                    wo_sb = wop.tile([128, NDC, D], BF16, name="wo_sb")
                    for dc in range(NDC):
                        nc.sync.dma_start(out=wo_sb[:, dc, :],
                                          in_=wo[li, 128 * dc:128 * (dc + 1), :])
                    qbs = range(8) if li == 0 else [7]
                    for qb in qbs:
                        h_tile = dpool.tile([128, D], F32, name="h_res")
                        if li == 0:
                            nc.sync.dma_start(
                                out=h_tile[:],
                                in_=h0[128 * qb:128 * (qb + 1), :])
                        else:
                            nc.sync.dma_start(
                                out=h_tile[:],
                                in_=agin1[128 * qb:128 * (qb + 1), :])
                        hn = dpool.tile([128, D], F32, name="hn")
                        for half in range(2):
                            ops = psD.tile([128, 512], F32, name="ops", bufs=2)
                            oT_cols = (slice(128 * qb, 128 * (qb + 1))
                                       if li == 0 else slice(0, 128))
                            for dc in range(NDC):
                                nc.tensor.matmul(
                                    ops[:],
                                    lhsT=oT[:, dc, oT_cols],
                                    rhs=wo_sb[:, dc, 512 * half:512 * (half + 1)],
                                    start=(dc == 0), stop=(dc == NDC - 1))
                            nc.vector.scalar_tensor_tensor(
                                out=hn[:, 512 * half:512 * (half + 1)],
                                in0=ops[:], scalar=1.0,
                                in1=h_tile[:, 512 * half:512 * (half + 1)],
                                op0=Alu.mult, op1=Alu.add)
                            nc.vector.tensor_add(
                                out=hn[:, 512 * half:512 * (half + 1)],
                                in0=hn[:, 512 * half:512 * (half + 1)],
                                in1=bo_sb[:, 512 * half:512 * (half + 1)])
                        if li == 0:
                            nc.sync.dma_start(
                                out=agin1[128 * qb:128 * (qb + 1), :],
                                in_=hn[:])
                        else:
                            nc.sync.dma_start(out=agin2[0:1, :],
                                              in_=hn[127:128, :])

            if li == 0:
                nc.gpsimd.collective_compute(
                    "AllGather", Alu.bypass,
                    replica_groups=[list(range(NCORES))],
                    ins=[agin1.opt()], outs=[agout1.opt()])
                if DEBUG:
                    nc.sync.dma_start(out=dbg_h1own[:], in_=agin1[:])
            else:
                nc.gpsimd.collective_compute(
                    "AllGather", Alu.bypass,
                    replica_groups=[list(range(NCORES))],
                    ins=[agin2.opt()], outs=[agout2.opt()])

        # ---------- head: final LN + vocab-sharded projection ----------
        with tc.tile_pool(name="hd", bufs=1) as hd, \
             tc.tile_pool(name="hdw", bufs=2) as hdw:
            x4 = hd.tile([4, D], F32)
            agout2_last = agout2s[repeats - 1]
            for i in range(4):
                nc.sync.dma_start(out=x4[i:i + 1, :],
                                  in_=agout2_last[2 * i:2 * i + 1, :])
            if DEBUG:
                nc.sync.dma_start(out=dbg_x4[:], in_=x4[:])
            xnf = hd.tile([4, D], F32)
            layernorm_tile(hd, x4, xnf, p=4)
            lnfg_sb = hd.tile([4, D], F32)
            nc.sync.dma_start(out=lnfg_sb[:], in_=lnfg[:])
            lnfb_sb = hd.tile([4, D], F32)
            nc.sync.dma_start(out=lnfb_sb[:], in_=lnfb[:])
            nc.vector.tensor_mul(out=xnf[:], in0=xnf[:], in1=lnfg_sb[:])
            nc.vector.tensor_add(out=xnf[:], in0=xnf[:], in1=lnfb_sb[:])

            ident_sb = hd.tile([128, 128], F32)
            nc.sync.dma_start(out=ident_sb[:], in_=ident[:])
            xhT = hd.tile([128, NDC, 4], BF16)
            with tc.tile_pool(name="psT", bufs=1, space="PSUM") as psT:
                for dc in range(NDC):
                    tps = psT.tile([128, 4], F32, name="tps", bufs=2)
                    nc.tensor.transpose(out=tps[:],
                                        in_=xnf[:, 128 * dc:128 * (dc + 1)],
                                        identity=ident_sb[0:4, 0:4])
                    nc.vector.tensor_copy(out=xhT[:, dc, :], in_=tps[:])

            headb_sb = hd.tile([4, CSH], F32)
            nc.sync.dma_start(out=headb_sb[:], in_=headb[:])
            with tc.tile_pool(name="psL", bufs=1, space="PSUM") as psL:
                lps = [psL.tile([4, 500], F32, name=f"lps{nb}")
                       for nb in range(8)]
                for dc in range(NDC):
                    hw = hdw.tile([128, CSH], BF16, name="hw")
                    nc.sync.dma_start(out=hw[:],
                                      in_=headw[128 * dc:128 * (dc + 1), :])
                    for nb in range(8):
                        nc.tensor.matmul(lps[nb][:], lhsT=xhT[:, dc, :],
                                         rhs=hw[:, 500 * nb:500 * (nb + 1)],
                                         start=(dc == 0), stop=(dc == NDC - 1))
                for nb in range(8):
                    lsb = hdw.tile([4, 500], F32, name="lsb")
                    nc.vector.tensor_add(out=lsb[:], in0=lps[nb][:],
                                         in1=headb_sb[:, 500 * nb:500 * (nb + 1)])
                    nc.sync.dma_start(out=logits[:, 500 * nb:500 * (nb + 1)],
                                      in_=lsb[:])

    nc.compile()
    return nc


def get_nc(repeats=1):
    key = f"nc{repeats}"
    if key not in _CACHE:
        _CACHE[key] = _build(repeats)
    return _CACHE[key]


def make_runner(in_maps, repeats=1):
    """Returns run_once() -> (out_arrs, wall_seconds) with device-cached
    inputs and a pre-traced executable (mirrors bass2jax.run_bass_via_pjrt)."""
    import time as _time
    import jax
    from jax.sharding import Mesh, PartitionSpec, NamedSharding
    from jax.experimental.shard_map import shard_map
    from concourse import bass2jax

    nc = get_nc(repeats)
    bass2jax.install_neuronx_cc_hook()
    partition_name = (nc.partition_id_tensor.name
                      if nc.partition_id_tensor else None)
    in_names, out_names, out_avals, zero_outs = [], [], [], []
    for alloc in nc.m.functions[0].allocations:
        if not isinstance(alloc, mybir.MemoryLocationSet):
            continue
        name = alloc.memorylocations[0].name
        if alloc.kind == "ExternalInput":
            if name != partition_name:
                in_names.append(name)
        elif alloc.kind == "ExternalOutput":
            shape = tuple(alloc.tensor_shape)
            dtype = mybir.dt.np(alloc.dtype)
            out_names.append(name)
            out_avals.append(jax.core.ShapedArray(shape, dtype))
            zero_outs.append(np.zeros(shape, dtype))
    n_params, n_outs = len(in_names), len(out_names)
    all_in = list(in_names) + list(out_names)
    if partition_name:
        all_in.append(partition_name)

    def _body(*args):
        operands = list(args)
        if partition_name:
            operands.append(bass2jax.partition_id_tensor())
        outs = bass2jax._bass_exec_p.bind(
            *operands, out_avals=tuple(out_avals), in_names=tuple(all_in),
            out_names=tuple(out_names), lowering_input_output_aliases=(),
            sim_require_finite=True, sim_require_nnan=True, nc=nc)
        return tuple(outs)

    devices = jax.devices()[:NCORES]
    mesh = Mesh(np.asarray(devices), ("core",))
    in_specs = (PartitionSpec("core"),) * (n_params + n_outs)
    out_specs = (PartitionSpec("core"),) * n_outs
    donate = tuple(range(n_params, n_params + n_outs))
    sharded = jax.jit(shard_map(_body, mesh=mesh, in_specs=in_specs,
                                out_specs=out_specs, check_rep=False),
                      donate_argnums=donate, keep_unused=True)
    sh = NamedSharding(mesh, PartitionSpec("core"))
    dev_in = [jax.device_put(
        np.concatenate([np.asarray(in_maps[c][k]) for c in range(NCORES)], 0), sh)
        for k in in_names]

    def run_once():
        dz = [jax.device_put(
            np.zeros((NCORES * z.shape[0], *z.shape[1:]), z.dtype), sh)
            for z in zero_outs]
        t0 = _time.time()
        out = sharded(*dev_in, *dz)
        jax.block_until_ready(out)
        dt = _time.time() - t0
        return dict(zip(out_names, out)), dt

    return run_once


def run_spmd(in_maps):
    nc = get_nc()
    return bass_utils.run_bass_kernel_spmd(nc, in_maps, core_ids=list(range(NCORES)))


def kernel(**inputs) -> np.ndarray:
    in_maps = _host_prep(inputs)
    res = run_spmd(in_maps)
    out = np.empty((B, C), np.float32)
    for c in range(NCORES):
        out[:, CSH * c:CSH * (c + 1)] = res.results[c]["logits"]
    _CACHE["last_results"] = res
    return out
